# revision 1
# baseline (speedup 1.0000x reference)
import numpy as np
import concourse.bass as bass
import concourse.bacc as bacc
import concourse.mybir as mybir
import concourse.tile as tile
from concourse.bass_utils import run_bass_kernel_spmd

N = 100000
E = 1600000
D = 128
NCORES = 8
NPC = 12544            # nodes per core
WPC = 98               # windows of 128 nodes per core
NPAD = NCORES * NPC    # 100352
NW = NCORES * WPC      # 784 global windows
LN_EPS = 1e-5

f32 = mybir.dt.float32
i32 = mybir.dt.int32
AF = mybir.ActivationFunctionType
OP = mybir.AluOpType
AX = mybir.AxisListType


def _edge_arrays(dst, src):
    """Sort edges by destination window; return per-core gather indices and
    local dst ids laid out [128, TC] (edge j of a window at p=j%128,
    c=coff[w]+j//128), plus per-window chunk counts."""
    wid = dst >> 7
    order = np.argsort(wid, kind="stable")
    ws = wid[order].astype(np.int64)
    ss = src[order].astype(np.int32)
    dl = (dst[order] & 127).astype(np.int32)
    counts = np.bincount(wid, minlength=NW)
    cw = (counts + 127) >> 7                         # chunks per (core, window)
    cw_w = cw.reshape(NCORES, WPC).max(axis=0)       # program chunk count per window
    cw_w = np.maximum(cw_w, 1)
    coff = np.zeros(WPC + 1, np.int64)
    np.cumsum(cw_w, out=coff[1:])
    TC = int(coff[-1])
    idx_arr = np.full((NCORES, 128, TC), N, np.int32)  # pad -> zero row of x_pad
    dl_arr = np.zeros((NCORES, 128, TC), np.int32)
    starts = np.zeros(NW, np.int64)
    np.cumsum(counts[:-1], out=starts[1:])
    rank = np.arange(len(ss), dtype=np.int64) - np.repeat(starts, counts)
    core = ws // WPC
    w = ws % WPC
    c = coff[w] + (rank >> 7)
    p = rank & 127
    idx_arr[core, p, c] = ss
    dl_arr[core, p, c] = dl
    return idx_arr, dl_arr, [int(v) for v in cw_w], [int(v) for v in coff[:-1]], TC


def _build_program(cw1, off1, TC1, cw2, off2, TC2, repeat=1):
    CM = max(max(cw1), max(cw2))
    nc = bacc.Bacc("TRN2", target_bir_lowering=False, debug=False)
    dp = nc.declare_dram_parameter
    x_in = dp("x", [NPAD, D], f32, isOutput=False)
    xo_in = dp("xo", [NPC, D], f32, isOutput=False)
    i1_in = dp("i1", [128, TC1], i32, isOutput=False)
    d1_in = dp("d1", [128, TC1], i32, isOutput=False)
    i2_in = dp("i2", [128, TC2], i32, isOutput=False)
    d2_in = dp("d2", [128, TC2], i32, isOutput=False)
    w1t_in = dp("w1t", [D, D], f32, isOutput=False)
    w2t_in = dp("w2t", [D, D], f32, isOutput=False)
    wl1a_in = dp("wl1a", [D, 2 * D], f32, isOutput=False)
    wl1b_in = dp("wl1b", [D, 2 * D], f32, isOutput=False)
    wl2a_in = dp("wl2a", [D, D], f32, isOutput=False)
    wl2b_in = dp("wl2b", [D, D], f32, isOutput=False)
    cn_in = {}
    for nm in ("g1b", "bt1b", "b1b", "g2b", "bt2b", "b2b", "bl2b", "iota", "ident"):
        cn_in[nm] = dp(nm, [128, D], f32, isOutput=False)
    cn_in["bl1b"] = dp("bl1b", [128, 2 * D], f32, isOutput=False)
    y_out = dp("y", [NPC, D], f32, isOutput=True)

    with tile.TileContext(nc) as tc:
        with tc.tile_pool(name="cst", bufs=1) as cst, \
             tc.tile_pool(name="io", bufs=2) as io, \
             tc.tile_pool(name="wk", bufs=2) as wk, \
             tc.tile_pool(name="psA", bufs=2, space="PSUM") as psA, \
             tc.tile_pool(name="psB", bufs=2, space="PSUM") as psB, \
             tc.tile_pool(name="psC", bufs=2, space="PSUM") as psC, \
             tc.tile_pool(name="psM", bufs=1, space="PSUM") as psM, \
             tc.tile_pool(name="psO", bufs=1, space="PSUM") as psO:

            def ld(name, param, shape):
                t = cst.tile(shape, f32, tag=name)
                nc.sync.dma_start(out=t[:], in_=param[:])
                return t

            zs = cst.tile([128, 1], f32, tag="zs")
            nc.vector.memset(zs[:], 0.0)
            eps = cst.tile([128, 1], f32, tag="eps")
            nc.vector.memset(eps[:], LN_EPS)
            nc.const_aps.aps[(f32, 0.0)] = zs[:]
            nc.const_aps.aps[(f32, LN_EPS)] = eps[:]

            # warm-up dummy gather (first indirect DMA in a program can be
            # flaky after malformed ones; cheap insurance)
            zidx = cst.tile([128, 1], i32, tag="zidx")
            nc.vector.memset(zidx[:], 0.0)
            scr = cst.tile([128, D], f32, tag="scr")
            nc.gpsimd.indirect_dma_start(
                out=scr[:], out_offset=None, in_=x_in[:],
                in_offset=bass.IndirectOffsetOnAxis(ap=zidx[:], axis=0))

            w1t = ld("w1t", w1t_in, [D, D])
            w2t = ld("w2t", w2t_in, [D, D])
            wl1a = ld("wl1a", wl1a_in, [D, 2 * D])
            wl1b = ld("wl1b", wl1b_in, [D, 2 * D])
            wl2a = ld("wl2a", wl2a_in, [D, D])
            wl2b = ld("wl2b", wl2b_in, [D, D])
            cn = {nm: ld(nm, cn_in[nm], [128, 2 * D] if nm == "bl1b" else [128, D])
                  for nm in cn_in}
            iota = cn["iota"]
            ident = cn["ident"]

            def window(w):
                CW1, CW2 = cw1[w], cw2[w]
                idx1 = io.tile([128, CM], i32, tag="idx1")
                nc.sync.dma_start(out=idx1[:, :CW1], in_=i1_in[:, off1[w]:off1[w] + CW1])
                dl1 = io.tile([128, CM], i32, tag="dl1")
                nc.sync.dma_start(out=dl1[:, :CW1], in_=d1_in[:, off1[w]:off1[w] + CW1])
                idx2 = io.tile([128, CM], i32, tag="idx2")
                nc.sync.dma_start(out=idx2[:, :CW2], in_=i2_in[:, off2[w]:off2[w] + CW2])
                dl2 = io.tile([128, CM], i32, tag="dl2")
                nc.sync.dma_start(out=dl2[:, :CW2], in_=d2_in[:, off2[w]:off2[w] + CW2])
                xw = io.tile([128, D], f32, tag="xw")
                nc.sync.dma_start(out=xw[:], in_=xo_in[w * 128:(w + 1) * 128, :])
                xg = []
                for b, (CWb, idxb) in enumerate(((CW1, idx1), (CW2, idx2))):
                    xgb = io.tile([128, CM * D], f32, tag=f"xg{b}")
                    for c in range(CWb):
                        nc.gpsimd.indirect_dma_start(
                            out=xgb[:, c * D:(c + 1) * D],
                            out_offset=None,
                            in_=x_in[:],
                            in_offset=bass.IndirectOffsetOnAxis(
                                ap=idxb[:, c:c + 1], axis=0),
                        )
                    xg.append(xgb)

                rt = []
                for b, (CWb, dlb, xgb, wbt, gB, btB, bB) in enumerate((
                        (CW1, dl1, xg[0], w1t, cn["g1b"], cn["bt1b"], cn["b1b"]),
                        (CW2, dl2, xg[1], w2t, cn["g2b"], cn["bt2b"], cn["b2b"]))):
                    dlf = wk.tile([128, CM], f32, tag=f"dlf{b}")
                    nc.vector.tensor_copy(out=dlf[:, :CWb], in_=dlb[:, :CWb])
                    hps = psA.tile([128, D], f32, tag="hps")
                    nc.tensor.matmul(out=hps[:], lhsT=ident[:], rhs=xw[:],
                                     start=True, stop=False)
                    for c in range(CWb):
                        oh = wk.tile([128, 128], f32, tag=f"oh{b}")
                        nc.vector.tensor_scalar(
                            out=oh[:], in0=iota[:],
                            scalar1=dlf[:, c:c + 1], scalar2=None,
                            op0=OP.is_equal)
                        nc.tensor.matmul(out=hps[:], lhsT=oh[:],
                                         rhs=xgb[:, c * D:(c + 1) * D],
                                         start=False, stop=(c == CWb - 1))
                    h_sb = wk.tile([128, D], f32, tag=f"h{b}")
                    nc.scalar.activation(out=h_sb[:], in_=hps[:], func=AF.Copy)
                    tps = psB.tile([128, D], f32, tag="tps")
                    nc.tensor.transpose(out=tps[:], in_=h_sb[:], identity=ident[:])
                    ht = wk.tile([128, D], f32, tag=f"ht{b}")
                    nc.vector.tensor_copy(out=ht[:], in_=tps[:])
                    zps = psC.tile([128, D], f32, tag="zps")
                    nc.tensor.matmul(out=zps[:], lhsT=ht[:], rhs=wbt[:],
                                     start=True, stop=True)
                    zb = wk.tile([128, D], f32, tag=f"zb{b}")
                    nc.vector.tensor_tensor(out=zb[:], in0=zps[:], in1=bB[:], op=OP.add)
                    musum = wk.tile([128, 1], f32, tag="musum")
                    nc.vector.tensor_reduce(out=musum[:], in_=zb[:], axis=AX.X, op=OP.add)
                    mu = wk.tile([128, 1], f32, tag="mu")
                    nc.scalar.activation(out=mu[:], in_=musum[:], func=AF.Copy,
                                         scale=1.0 / D)
                    zc = wk.tile([128, D], f32, tag=f"zc{b}")
                    nc.vector.tensor_scalar(out=zc[:], in0=zb[:], scalar1=mu[:],
                                            scalar2=None, op0=OP.subtract)
                    sq = wk.tile([128, D], f32, tag="sq")
                    vsum = wk.tile([128, 1], f32, tag="vsum")
                    nc.scalar.activation(out=sq[:], in_=zc[:], func=AF.Square,
                                         accum_out=vsum[:])
                    std = wk.tile([128, 1], f32, tag="std")
                    nc.scalar.activation(out=std[:], in_=vsum[:], func=AF.Sqrt,
                                         scale=1.0 / D, bias=LN_EPS)
                    rs = wk.tile([128, 1], f32, tag="rs")
                    nc.vector.reciprocal(out=rs[:], in_=std[:])
                    zn = wk.tile([128, D], f32, tag=f"zn{b}")
                    nc.vector.tensor_scalar(out=zn[:], in0=zc[:], scalar1=rs[:],
                                            scalar2=None, op0=OP.mult)
                    yg = wk.tile([128, D], f32, tag=f"yg{b}")
                    nc.vector.tensor_tensor(out=yg[:], in0=zn[:], in1=gB[:], op=OP.mult)
                    ya = wk.tile([128, D], f32, tag=f"ya{b}")
                    nc.gpsimd.tensor_tensor(out=ya[:], in0=yg[:], in1=btB[:], op=OP.add)
                    yv = wk.tile([128, D], f32, tag=f"yv{b}")
                    nc.scalar.activation(out=yv[:], in_=ya[:], func=AF.Relu)
                    ret = wk.tile([128, D], f32, tag=f"ret{b}")
                    nc.gpsimd.tensor_tensor(out=ret[:], in0=h_sb[:], in1=yv[:], op=OP.add)
                    rtp = psB.tile([128, D], f32, tag="tps")
                    nc.tensor.transpose(out=rtp[:], in_=ret[:], identity=ident[:])
                    rtb = wk.tile([128, D], f32, tag=f"rt{b}")
                    nc.vector.tensor_copy(out=rtb[:], in_=rtp[:])
                    rt.append(rtb)

                mps = psM.tile([128, 2 * D], f32, tag="mps")
                nc.tensor.matmul(out=mps[:], lhsT=rt[0][:], rhs=wl1a[:],
                                 start=True, stop=False)
                nc.tensor.matmul(out=mps[:], lhsT=rt[1][:], rhs=wl1b[:],
                                 start=False, stop=True)
                mb = wk.tile([128, 2 * D], f32, tag="mb")
                nc.vector.tensor_tensor(out=mb[:], in0=mps[:], in1=cn["bl1b"][:], op=OP.add)
                mr = wk.tile([128, 2 * D], f32, tag="mr")
                nc.scalar.activation(out=mr[:], in_=mb[:], func=AF.Relu)
                mta_ps = psB.tile([128, D], f32, tag="tps")
                nc.tensor.transpose(out=mta_ps[:], in_=mr[:, :D], identity=ident[:])
                mta = wk.tile([128, D], f32, tag="mta")
                nc.vector.tensor_copy(out=mta[:], in_=mta_ps[:])
                mtb_ps = psB.tile([128, D], f32, tag="tps")
                nc.tensor.transpose(out=mtb_ps[:], in_=mr[:, D:], identity=ident[:])
                mtb = wk.tile([128, D], f32, tag="mtb")
                nc.scalar.activation(out=mtb[:], in_=mtb_ps[:], func=AF.Copy)
                ops_t = psO.tile([128, D], f32, tag="ops")
                nc.tensor.matmul(out=ops_t[:], lhsT=mta[:], rhs=wl2a[:],
                                 start=True, stop=False)
                nc.tensor.matmul(out=ops_t[:], lhsT=mtb[:], rhs=wl2b[:],
                                 start=False, stop=True)
                ob = wk.tile([128, D], f32, tag="ob")
                nc.vector.tensor_tensor(out=ob[:], in0=ops_t[:], in1=cn["bl2b"][:], op=OP.add)
                orl = wk.tile([128, D], f32, tag="orl")
                nc.scalar.activation(out=orl[:], in_=ob[:], func=AF.Relu)
                nc.sync.dma_start(out=y_out[w * 128:(w + 1) * 128, :], in_=orl[:])

            def body():
                for w in range(WPC):
                    window(w)

            if repeat > 1:
                with tc.For_i(0, repeat, 1):
                    body()
            else:
                body()
    nc.finalize()
    return nc


def prepare(x, ei, W1, b1, g1, bt1, W2, b2, g2, bt2, Wl1, bl1, Wl2, bl2,
            repeat=1):
    """Build (nc, in_maps) without running."""
    x = np.ascontiguousarray(np.asarray(x, np.float32))
    ei = np.asarray(ei, np.int64)
    x_pad = np.zeros((NPAD, D), np.float32)
    x_pad[:N] = x
    src, dst = ei[0], ei[1]
    i1a, d1a, cw1, off1, TC1 = _edge_arrays(dst, src)
    i2a, d2a, cw2, off2, TC2 = _edge_arrays(src, dst)
    bc = lambda v, n=D: np.ascontiguousarray(
        np.broadcast_to(np.asarray(v, np.float32), (128, n)))
    common = {
        "x": x_pad,
        "w1t": np.ascontiguousarray(np.asarray(W1, np.float32).T),
        "w2t": np.ascontiguousarray(np.asarray(W2, np.float32).T),
        "wl1a": np.ascontiguousarray(np.asarray(Wl1, np.float32).T[:D]),
        "wl1b": np.ascontiguousarray(np.asarray(Wl1, np.float32).T[D:]),
        "wl2a": np.ascontiguousarray(np.asarray(Wl2, np.float32).T[:D]),
        "wl2b": np.ascontiguousarray(np.asarray(Wl2, np.float32).T[D:]),
        "g1b": bc(g1), "bt1b": bc(bt1), "b1b": bc(b1),
        "g2b": bc(g2), "bt2b": bc(bt2), "b2b": bc(b2),
        "bl1b": bc(bl1, 2 * D), "bl2b": bc(bl2),
        "iota": bc(np.arange(D, dtype=np.float32)),
        "ident": np.eye(128, dtype=np.float32),
    }
    in_maps = []
    for k in range(NCORES):
        m = dict(common)
        m["xo"] = np.ascontiguousarray(x_pad[k * NPC:(k + 1) * NPC])
        m["i1"] = np.ascontiguousarray(i1a[k])
        m["d1"] = np.ascontiguousarray(d1a[k])
        m["i2"] = np.ascontiguousarray(i2a[k])
        m["d2"] = np.ascontiguousarray(d2a[k])
        in_maps.append(m)
    nc = _build_program(cw1, off1, TC1, cw2, off2, TC2, repeat=repeat)
    return nc, in_maps


def kernel(x, ei, W1, b1, g1, bt1, W2, b2, g2, bt2, Wl1, bl1, Wl2, bl2,
           _repeat=1, _timing=None):
    x = np.ascontiguousarray(np.asarray(x, np.float32))
    ei = np.asarray(ei, np.int64)
    x_pad = np.zeros((NPAD, D), np.float32)
    x_pad[:N] = x
    src, dst = ei[0], ei[1]
    i1a, d1a, cw1, off1, TC1 = _edge_arrays(dst, src)   # branch1: agg over dst
    i2a, d2a, cw2, off2, TC2 = _edge_arrays(src, dst)   # branch2: flipped

    bc = lambda v, n=D: np.ascontiguousarray(
        np.broadcast_to(np.asarray(v, np.float32), (128, n)))
    common = {
        "x": x_pad,
        "w1t": np.ascontiguousarray(np.asarray(W1, np.float32).T),
        "w2t": np.ascontiguousarray(np.asarray(W2, np.float32).T),
        "wl1a": np.ascontiguousarray(np.asarray(Wl1, np.float32).T[:D]),
        "wl1b": np.ascontiguousarray(np.asarray(Wl1, np.float32).T[D:]),
        "wl2a": np.ascontiguousarray(np.asarray(Wl2, np.float32).T[:D]),
        "wl2b": np.ascontiguousarray(np.asarray(Wl2, np.float32).T[D:]),
        "g1b": bc(g1), "bt1b": bc(bt1), "b1b": bc(b1),
        "g2b": bc(g2), "bt2b": bc(bt2), "b2b": bc(b2),
        "bl1b": bc(bl1, 2 * D), "bl2b": bc(bl2),
        "iota": bc(np.arange(D, dtype=np.float32)),
        "ident": np.eye(128, dtype=np.float32),
    }
    in_maps = []
    for k in range(NCORES):
        m = dict(common)
        m["xo"] = np.ascontiguousarray(x_pad[k * NPC:(k + 1) * NPC])
        m["i1"] = np.ascontiguousarray(i1a[k])
        m["d1"] = np.ascontiguousarray(d1a[k])
        m["i2"] = np.ascontiguousarray(i2a[k])
        m["d2"] = np.ascontiguousarray(d2a[k])
        in_maps.append(m)

    nc = _build_program(cw1, off1, TC1, cw2, off2, TC2, repeat=_repeat)
    res = run_bass_kernel_spmd(nc, in_maps, list(range(NCORES)))
    if _timing is not None:
        import time
        for _ in range(int(_timing)):
            t0 = time.time()
            res = run_bass_kernel_spmd(nc, in_maps, list(range(NCORES)))
            _timing_walls.append(time.time() - t0)
    out = np.concatenate([res.results[k]["y"] for k in range(NCORES)], axis=0)
    return np.ascontiguousarray(out[:N])


_timing_walls = []

